# revision 1
# baseline (speedup 1.0000x reference)
"""Multi-head self-attention (B=2, S=2048, D=1024, H=16, causal+padding mask)
on 8 Trainium2 NeuronCores via Bass/Tile, SPMD.

Sharding: core c -> batch b = c//4, query residue r = c%4. Each core computes
the full K/V projections for its batch (duplicated across the 4 cores of a
batch -- cheaper than any cross-core collective at this size) and attention +
output projection for the strided query set q = 4j + r, j = 0..511. Strided
(rather than blocked) query assignment keeps the causal loop structure
identical on every core, which SPMD requires; per-core differences (mask
values, packed activations) travel as data.

Dataflow is fully transposed so no on-chip transposes are needed:
  QT[dh, q]   = (Wq x^T)/8 + bq/8          lhsT = Wq^T chunks, rhs = xq^T
  KT[dh, k]   = Wk x^T + bk
  V [k, dh]   = x Wv^T  (+ ones column)    lhsT = x^T chunks,  rhs = Wv^T
  ST[k, q]    = KT_h^T QT_h  (per head)
  E           = exp(ST + padmask_bias) * causal01
  OT'[dh+1,q] = V_aug^T E   (row 64 = softmax denominators r[q])
  AT[dh, q]   = OT' * (1/r)                (broadcast r via rank-1 matmul,
                                            then reciprocal on all 64 lanes)
  YT[n, q]    = Wo_arr^T AT + (bo + Wo bv) lhsT = Wo^T chunks, rhs = AT
Matmuls run in float32r (full-rate fp32 with reduced mantissa, ~1e-4 rel
error). Softmax skips max-subtraction: scores are bounded (|S| < ~5) so exp
is safe, and masked lanes get -1e4 added pre-exp which underflows to exactly
0 after exp.
"""

import sys

if "/opt/trn_rl_repo" not in sys.path:
    sys.path.insert(0, "/opt/trn_rl_repo")

import numpy as np

B, S, D, H, HD = 2, 2048, 1024, 16, 64
N_CORES = 8
JQ = S // 4          # 512 packed queries per core
MC = D // 128        # 8 contraction chunks of 128
NKT = S // 128       # 16 key tiles
JB_N = 256           # packed query block (matmul N)

_CACHE = {}


def _split_waits(nc, mybir):
    """This walrus build accepts only one sync-wait per instruction; move
    extra waits onto NOPs inserted just before, on the same engine."""
    n_new = 0
    for f in nc.m.functions:
        for blk in f.blocks:
            out = []
            for inst in blk.instructions:
                si = inst.sync_info
                if si is not None and si.on_wait is not None and len(si.on_wait) > 1:
                    waits = list(si.on_wait)
                    for w in waits[:-1]:
                        n_new += 1
                        out.append(mybir.InstNoOp(
                            name=f"I-waitsplit-{n_new}",
                            engine=inst.engine,
                            ins=[], outs=[],
                            sync_info=mybir.SyncInfo(on_wait=[w], on_update=[]),
                        ))
                    inst.sync_info = mybir.SyncInfo(
                        on_wait=[waits[-1]], on_update=list(si.on_update or []))
                out.append(inst)
            blk.instructions[:] = out
    return n_new


def _build():
    import concourse.bass as bass
    import concourse.mybir as mybir
    import concourse.tile as tile
    from contextlib import ExitStack

    f32 = mybir.dt.float32
    f32r = mybir.dt.float32r
    EXP = mybir.ActivationFunctionType.Exp
    IDENT = mybir.ActivationFunctionType.Identity
    COPY = mybir.ActivationFunctionType.Copy

    nc = bass.Bass()
    xT = nc.declare_dram_parameter("xT", [D, S], f32r, isOutput=False)
    xqT = nc.declare_dram_parameter("xqT", [D, JQ], f32r, isOutput=False)
    wqT = nc.declare_dram_parameter("wqT", [D, D], f32r, isOutput=False)
    wkT = nc.declare_dram_parameter("wkT", [D, D], f32r, isOutput=False)
    wvT = nc.declare_dram_parameter("wvT", [D, D], f32r, isOutput=False)
    woT = nc.declare_dram_parameter("woT", [D, D], f32r, isOutput=False)
    bq8 = nc.declare_dram_parameter("bq8", [D], f32, isOutput=False)
    bkv = nc.declare_dram_parameter("bk", [D], f32, isOutput=False)
    obias = nc.declare_dram_parameter("obias", [D], f32, isOutput=False)
    pmb = nc.declare_dram_parameter("pmb", [S], f32, isOutput=False)
    cmask = nc.declare_dram_parameter("cmask", [8, 128, JB_N], f32r, isOutput=False)
    onesc = nc.declare_dram_parameter("onesc", [1, HD], f32r, isOutput=False)
    out = nc.declare_dram_parameter("o", [D, JQ], f32, isOutput=True)

    with tile.TileContext(nc) as tc, ExitStack() as ctx, \
            nc.allow_low_precision("fp32r matmul inputs keep ~19 bits"):
        ec = ctx.enter_context
        consts = ec(tc.tile_pool(name="consts", bufs=1))
        big = ec(tc.tile_pool(name="big", bufs=1))
        e_p = ec(tc.tile_pool(name="e", bufs=6))
        rc_p = ec(tc.tile_pool(name="rc", bufs=1))
        rb_p = ec(tc.tile_pool(name="rb", bufs=1))
        yt_p = ec(tc.tile_pool(name="yt", bufs=2))
        proj_ps = ec(tc.tile_pool(name="proj_ps", bufs=2, space="PSUM"))
        st_ps = ec(tc.tile_pool(name="st_ps", bufs=3, space="PSUM"))
        ot_ps = ec(tc.tile_pool(name="ot_ps", bufs=3, space="PSUM"))

        # ---- constants into SBUF ----
        bq8_sb = consts.tile([128, MC], f32, tag="bq8")
        nc.sync.dma_start(out=bq8_sb, in_=bq8.rearrange("(c p) -> p c", p=128))
        bk_sb = consts.tile([128, MC], f32, tag="bk")
        nc.sync.dma_start(out=bk_sb, in_=bkv.rearrange("(c p) -> p c", p=128))
        ob_sb = consts.tile([128, MC], f32, tag="ob")
        nc.sync.dma_start(out=ob_sb, in_=obias.rearrange("(c p) -> p c", p=128))
        pmb_sb = consts.tile([128, NKT], f32, tag="pmb")
        nc.sync.dma_start(out=pmb_sb, in_=pmb.rearrange("(t p) -> p t", p=128))
        cm_sb = consts.tile([128, 8, JB_N], f32r, tag="cm")
        nc.sync.dma_start(out=cm_sb, in_=cmask.rearrange("t p j -> p t j"))
        ones_sb = consts.tile([1, HD], f32r, tag="ones")
        nc.sync.dma_start(out=ones_sb, in_=onesc[:, :])

        # persistent activations
        QT_sb = big.tile([128, MC, JQ], f32r, tag="qt")            # 16KB/part
        KT_sb = big.tile([128, MC, S], f32r, tag="kt")             # 64KB/part
        V_sb = big.tile([128, NKT, H, HD + 1], f32r, tag="v")      # 66.6KB/part
        # xq (Q-proj phase) and AT (attention/output phases) have disjoint
        # lifetimes; share one 16KB slot via a common tag.
        xq_sb = big.tile([128, MC, JQ], f32r, tag="xqat")
        MULT = mybir.AluOpType.mult
        ADD = mybir.AluOpType.add
        xre = xT.rearrange("(c p) k -> p c k", p=128)
        wkre = wkT.rearrange("(c p) n -> p c n", p=128)
        wvre = wvT.rearrange("(c p) n -> p c n", p=128)
        AT_sb = None

        def attention_pair(h0, jb, kt_lo, kt_hi, otps):
            """Emit S^T/exp/mask/PV for heads (h0, h0+1), query block jb,
            key tiles [kt_lo, kt_hi), interleaved for PE overlap. otps holds
            the two accumulation psum tiles (allocated at kt_lo==0)."""
            nkt = 8 if jb == 0 else 16
            for kt in range(kt_lo, kt_hi):
                for hi in range(2):
                    h = h0 + hi
                    pr, hw = h // 2, 64 * (h % 2)
                    st = st_ps.tile([128, JB_N], f32, tag="st")
                    nc.tensor.matmul(
                        st[:],
                        KT_sb[hw:hw + 64, pr, kt * 128:(kt + 1) * 128],
                        QT_sb[hw:hw + 64, pr, jb * JB_N:(jb + 1) * JB_N],
                        start=True, stop=True)
                    e = e_p.tile([128, JB_N], f32r, tag="e")
                    nc.scalar.activation(out=e[:], in_=st[:], func=EXP,
                                         bias=pmb_sb[:, kt:kt + 1])
                    tp = kt - 8 * jb
                    if tp >= 0:
                        nc.gpsimd.tensor_mul(e[:], e[:], cm_sb[:, tp, :])
                    nc.tensor.matmul(otps[hi][:], V_sb[:, kt, h, :], e[:],
                                     start=(kt == 0), stop=(kt == nkt - 1))

        def attention_norm(h0, jb, otps):
            for hi in range(2):
                h = h0 + hi
                pr, hw = h // 2, 64 * (h % 2)
                otp = otps[hi]
                rc = rc_p.tile([1, JB_N], f32r, tag="rc")
                nc.scalar.activation(out=rc[:], in_=otp[HD:HD + 1, :], func=COPY)
                bc = st_ps.tile([HD, JB_N], f32, tag="st")
                nc.tensor.matmul(bc[:], ones_sb[:], rc[:], start=True, stop=True)
                rb = rb_p.tile([HD, JB_N], f32, tag="rb")
                nc.vector.reciprocal(out=rb[:], in_=bc[:])
                nc.vector.tensor_mul(
                    AT_sb[hw:hw + 64, pr, jb * JB_N:(jb + 1) * JB_N],
                    otp[0:HD, :], rb[:])

        def attention(h0, kt_done):
            """Full attention for heads (h0, h0+1) given KT/V ready up to
            kt_done; emits both query blocks."""
            for jb in range(2):
                nkt = 8 if jb == 0 else 16
                otp_a = ot_ps.tile([HD + 1, JB_N], f32, tag="ot")
                otp_b = ot_ps.tile([HD + 1, JB_N], f32, tag="ot")
                otps = [otp_a, otp_b]
                attention_pair(h0, jb, 0, min(nkt, kt_done), otps)
                if kt_done < nkt:
                    attention_pair(h0, jb, kt_done, nkt, otps)
                attention_norm(h0, jb, otps)

        # ---- Q projection: QT[dh, jq] = Wq x^T /8 + bq/8 ----
        nc.sync.dma_start(out=xq_sb,
                          in_=xqT.rearrange("(c p) j -> p c j", p=128))
        with tc.tile_pool(name="wq", bufs=2) as wq_p:
            for q4 in range(4):
                wq_sb = wq_p.tile([128, MC, 256], f32r, tag="wq")
                nc.sync.dma_start(
                    out=wq_sb,
                    in_=wqT.rearrange("(c p) n -> p c n", p=128)[:, :, q4 * 256:(q4 + 1) * 256])
                for dt_i in range(2):
                    dt_ = q4 * 2 + dt_i
                    ps = proj_ps.tile([128, 512], f32, tag="ps")
                    for m in range(MC):
                        nc.tensor.matmul(
                            ps[:], wq_sb[:, m, dt_i * 128:(dt_i + 1) * 128],
                            xq_sb[:, m, :],
                            start=(m == 0), stop=(m == MC - 1))
                    nc.vector.tensor_scalar(
                        out=QT_sb[:, dt_, :], in0=ps[:],
                        scalar1=0.125, scalar2=bq8_sb[:, dt_:dt_ + 1],
                        op0=MULT, op1=ADD)

        AT_sb = big.tile([128, MC, JQ], f32r, tag="xqat")

        # ---- K/V projections split by dh half (head groups 0-7 / 8-15) so
        # attention on the first half overlaps the second half's projections.
        for half in range(2):
            # K rows for pairs [4*half, 4*half+4)
            with tc.tile_pool(name="kproj", bufs=1) as kp, \
                    tc.tile_pool(name="kw", bufs=2) as kwp:
                for kb8 in range(8):
                    xt_sb = kp.tile([128, MC, 256], f32r, tag="xt")
                    nc.sync.dma_start(
                        out=xt_sb, in_=xre[:, :, kb8 * 256:(kb8 + 1) * 256])
                    for q4 in (2 * half, 2 * half + 1):
                        wk_sb = kwp.tile([128, MC, 256], f32r, tag="wk")
                        nc.sync.dma_start(
                            out=wk_sb, in_=wkre[:, :, q4 * 256:(q4 + 1) * 256])
                        for dt_i in range(2):
                            dt_ = q4 * 2 + dt_i
                            ps = proj_ps.tile([128, 256], f32, tag="ps")
                            for m in range(MC):
                                nc.tensor.matmul(
                                    ps[:], wk_sb[:, m, dt_i * 128:(dt_i + 1) * 128],
                                    xt_sb[:, m, :],
                                    start=(m == 0), stop=(m == MC - 1))
                            nc.vector.tensor_scalar_add(
                                out=KT_sb[:, dt_, kb8 * 256:(kb8 + 1) * 256],
                                in0=ps[:], scalar1=bk_sb[:, dt_:dt_ + 1])
            # V columns for heads [8*half, 8*half+8)
            with tc.tile_pool(name="vproj", bufs=1) as vp, \
                    tc.tile_pool(name="vxt", bufs=2) as vxt:
                wv_sb = vp.tile([128, MC, 512], f32r, tag="wv")
                nc.sync.dma_start(
                    out=wv_sb, in_=wvre[:, :, half * 512:(half + 1) * 512])
                for kt in range(NKT):
                    xt_sb = vxt.tile([128, MC, 128], f32r, tag="xtv")
                    nc.sync.dma_start(
                        out=xt_sb, in_=xre[:, :, kt * 128:(kt + 1) * 128])
                    ps = proj_ps.tile([128, 512], f32, tag="ps")
                    for m in range(MC):
                        nc.tensor.matmul(
                            ps[:], xt_sb[:, m, :], wv_sb[:, m, :],
                            start=(m == 0), stop=(m == MC - 1))
                    nc.vector.tensor_copy(
                        V_sb[:, kt, half * 8:(half + 1) * 8, 0:HD],
                        ps[:].rearrange("p (h d) -> p h d", d=HD))
                    nc.vector.tensor_scalar(
                        out=V_sb[:, kt, half * 8:(half + 1) * 8, HD:HD + 1],
                        in0=ps[:].rearrange("p (h d) -> p h d", d=HD)[:, :, 0:1],
                        scalar1=0.0, scalar2=1.0, op0=MULT, op1=ADD)
            # attention for this half's heads (overlaps next half's K/V proj)
            for h0 in range(8 * half, 8 * half + 8, 2):
                attention(h0, NKT)

        # ---- output projection: YT[n, jq] ----
        with tc.tile_pool(name="oproj", bufs=2) as op:
            for q4 in range(4):
                wo_sb = op.tile([128, MC, 256], f32r, tag="wo")
                nc.sync.dma_start(
                    out=wo_sb,
                    in_=woT.rearrange("(c p) n -> p c n", p=128)[:, :, q4 * 256:(q4 + 1) * 256])
                for nt_i in range(2):
                    nt = q4 * 2 + nt_i
                    ps = proj_ps.tile([128, JQ], f32, tag="ps")
                    for c in range(MC):
                        nc.tensor.matmul(
                            ps[:], wo_sb[:, c, nt_i * 128:(nt_i + 1) * 128],
                            AT_sb[:, c, :],
                            start=(c == 0), stop=(c == MC - 1))
                    yt = yt_p.tile([128, JQ], f32, tag="yt")
                    nc.scalar.activation(out=yt[:], in_=ps[:], func=IDENT,
                                         bias=ob_sb[:, nt:nt + 1])
                    nc.sync.dma_start(out=out[nt * 128:(nt + 1) * 128, :], in_=yt[:])

    _split_waits(nc, mybir)
    return nc


def _get_nc():
    if "nc" not in _CACHE:
        _CACHE["nc"] = _build()
    return _CACHE["nc"]


def _make_inputs(x, mask, Wq, bq, Wk, bk, Wv, bv, Wo, bo):
    f = np.float32
    x = np.asarray(x, f)
    mask = np.asarray(mask)
    Wq, bq = np.asarray(Wq, f), np.asarray(bq, f)
    Wk, bk = np.asarray(Wk, f), np.asarray(bk, f)
    Wv, bv = np.asarray(Wv, f), np.asarray(bv, f)
    Wo, bo = np.asarray(Wo, f), np.asarray(bo, f)

    wqT = np.ascontiguousarray(Wq.T)
    wkT = np.ascontiguousarray(Wk.T)
    wvT = np.ascontiguousarray(Wv.T)
    woT = np.ascontiguousarray(Wo.T)
    bq8 = (bq / 8.0).astype(f)
    obias = (bo + Wo @ bv).astype(f)

    xTb = [np.ascontiguousarray(x[b].T) for b in range(B)]
    pmbb = [((mask[b].astype(f) - 1.0) * 1e4).astype(f) for b in range(B)]

    ii, jj = np.meshgrid(np.arange(128), np.arange(JB_N), indexing="ij")
    onesc = np.ones((1, HD), f)

    ins = []
    for c in range(N_CORES):
        b, r = c // 4, c % 4
        cm = np.empty((8, 128, JB_N), f)
        for tp in range(8):
            cm[tp] = (128 * tp + ii <= 4 * jj + r).astype(f)
        ins.append({
            "xT": xTb[b],
            "xqT": np.ascontiguousarray(x[b].T[:, r::4]),
            "wqT": wqT, "wkT": wkT, "wvT": wvT, "woT": woT,
            "bq8": bq8, "bk": bk, "obias": obias,
            "pmb": pmbb[b],
            "cmask": cm,
            "onesc": onesc,
        })
    return ins


def _run(ins, trace=False):
    from concourse.bass_utils import run_bass_kernel_spmd
    nc = _get_nc()
    return run_bass_kernel_spmd(nc, ins, list(range(N_CORES)), trace=trace)


def kernel(x, mask, Wq, bq, Wk, bk, Wv, bv, Wo, bo):
    ins = _make_inputs(x, mask, Wq, bq, Wk, bk, Wv, bv, Wo, bo)
    res = _run(ins)
    out = np.empty((B, S, D), np.float32)
    for c in range(N_CORES):
        b, r = c // 4, c % 4
        out[b, r::4, :] = res.results[c]["o"].T
    return out

